# revision 9
# baseline (speedup 1.0000x reference)
"""Contrastive-loss kernel for Trainium2, SPMD over 8 NeuronCores.

The reference loss over x[N=4, S=4096, F=256] is, for pairs a>b with
D[a,b] = ||x[:,a]-x[:,b]||^2 (summed over batch and feature):

    loss = [ sum_{a>b, a-b>1} D[a,b] + sum_{b} relu(M - D[b+1,b]) ] / (S*(S-1)*1000)

Using symmetry of D this collapses to a streaming computation:

    sum_{a>b} D = S * sum_t s[t] - sum_{n,f} c[n,f]^2
    s[t]        = sum_{n,f} x[n,t,f]^2,   c[n,f] = sum_t x[n,t,f]
    D_sub[b]    = ||x[:,b+1]-x[:,b]||^2
    numerator   = sum_{a>b} D - sum_b D_sub[b] + sum_b relu(M - D_sub[b])

Sharding: 512 sequence rows per core, loaded ONCE (no halo, no double
load) as two [128, N, 2F] tiles where partition p holds the row pair
(2p, 2p+1) as one contiguous 2KB DMA run.  Even-pair diffs are a DVE
subtract within the partition; odd-pair diffs use a PE shift-matmul
(float32r, full rate) to move row 2p+2 down to partition p.  Sum of
squares s comes from one fused DVE tensor_tensor_reduce per tile, and
the column sums c accumulate on the PE into one PSUM row.  The host
combines the 8 cores' partials in float64 and adds the 15 pair terms
that straddle a 256-row tile boundary (exact fixup, trivial numpy).
"""

import numpy as np

import concourse.bass as bass
import concourse.tile as tile
from concourse import mybir
from concourse.bass_utils import run_bass_kernel_spmd

N, S, F = 4, 4096, 256
F2 = 2 * F                     # 512 floats = one 2KB row pair
NF = N * F                     # 1024
NCORES = 8
LOCAL = S // NCORES            # 512 rows per core
ROWS = 256                     # rows per tile (128 partitions x 2)
TPC = LOCAL // ROWS            # 2 tiles per core
MARGIN = 60000.0
OUTW = NF + 16                 # c partial (1024) + fin [2, 10] at 1024:1034

_program = None
TRACE = False
LAST_RESULT = None


def _patch_sem_clear():
    """The walrus build in this container cannot encode
    EVENT_SEMAPHORE_RANGE_CLEAR ("ISA wrong length" in codegen). Replace the
    tail range-clear that TileContext emits via Bass.clear_and_free_semaphores
    with per-semaphore EventSemaphore writes of 0 (sem-wr-imm), which the
    compiler does support."""
    import bass_rust
    from concourse.bass import compact_to_ranges

    if getattr(bass.Bass, "_sem_clear_patched", False):
        return

    def clear_and_free_semaphores(self, sems):
        if not sems:
            return
        sem_nums = [s.num if hasattr(s, "num") else s for s in sems]
        for sem_range in compact_to_ranges(sem_nums):
            assert self._state.free_isdisjoint(sem_range)
            self.gpsimd.dma_reset(sem_range)
            for num in sem_range:
                h = bass_rust.SemaphoreHandle(num=num, name=f"clr{num}")
                bi = self.gpsimd.sem_inc(h, 1)
                upd = bi.ins.sync_info.on_update[0]
                upd.update_mode = "sem-wr-imm"
                upd.update_value = 0
        self._state.prepend_free_semaphores(sem_nums)
        for poison_set in self._tile_sem_poison_stack:
            poison_set.update(sem_nums)

    bass.Bass.clear_and_free_semaphores = clear_and_free_semaphores
    bass.Bass._sem_clear_patched = True


def _split_multi_waits(nc: bass.Bass) -> None:
    """The walrus build here encodes at most ONE sync wait per instruction.
    Hoist surplus waits into standalone wait-only EventSemaphore instructions
    placed immediately before the owner on the same engine queue — semantics
    are identical (same queue, in-order), and every instruction ends up with
    a single wait."""
    import bass_rust

    wid = 0
    for b in nc.m.functions[0].blocks:
        out = []
        changed = False
        for inst in b.instructions:
            si = inst.sync_info
            waits = list(si.on_wait) if si is not None else []
            if len(waits) > 1:
                changed = True
                for w in waits[:-1]:
                    ev = bass_rust.InstEventSemaphore(
                        name=f"WSPLIT-{wid}", engine=inst.engine, ins=[], outs=[]
                    )
                    wid += 1
                    ev.sync_info = bass_rust.SyncInfo(on_wait=[w], on_update=[])
                    out.append(ev)
                inst.sync_info = bass_rust.SyncInfo(
                    on_wait=[waits[-1]], on_update=list(si.on_update)
                )
            out.append(inst)
        if changed:
            b.instructions = out


def _build_program() -> bass.Bass:
    _patch_sem_clear()
    f32 = mybir.dt.float32
    bf16 = mybir.dt.bfloat16
    nc = bass.Bass()
    xc = nc.declare_dram_parameter("xc", [N, LOCAL, F], f32, isOutput=False)
    # inb cols: 0:128 shift matrix (col i selects row i+1; col 127 zero),
    # 128 ones, 129 mask (1s, [127]=0), 130 margin, 131 pad
    inb = nc.declare_dram_parameter("inb", [128, 132], f32, isOutput=False)
    out = nc.declare_dram_parameter("out", [2, OUTW], f32, isOutput=True)

    with tile.TileContext(nc) as tc:
        with (
            tc.tile_pool(name="data", bufs=2) as data,
            tc.tile_pool(name="scratch", bufs=2) as scratch,
            tc.tile_pool(name="small", bufs=1) as small,
            tc.tile_pool(name="psum", bufs=1, space="PSUM") as psum,
        ):
            # All loads go through the Pool engine's SWDGE ring, which is
            # the only one that can CAST on the fly: x streams from f32 DRAM
            # straight into bf16 SBUF tiles (no compute-engine cast pass).
            # Tile t covers rows [256t, 256t+255]; partition p holds the
            # contiguous 2KB pair (2p, 2p+1) per n.  The bf16 shift matrix
            # goes first (it's tiny and the PE needs it before tile 0).
            inbB = small.tile([128, 130], bf16)
            nc.gpsimd.dma_start(out=inbB, in_=inb[:, 0:130])
            P2s = []
            for t in range(TPC):
                P = data.tile([128, N, F2], bf16, tag="P2")
                P2s.append(P)
                src = bass.AP(
                    tensor=xc[:, :, :].tensor,
                    offset=t * ROWS * F,
                    ap=[[F2, 128], [LOCAL * F, N], [1, F2]],
                )
                nc.gpsimd.dma_start(out=P, in_=src)
            # f32 [ones, mask, margin] for the final reduction + hinge bias
            inbF = small.tile([128, 3], f32)
            nc.gpsimd.dma_start(out=inbF, in_=inb[:, 128:131])

            # Warm the activation table while the data loads: the first ACT
            # op pays a ~1.3us table load otherwise on the critical path.
            wsrc = small.tile([128, 1], f32)
            nc.vector.memset(wsrc, 1.0)
            wdst = small.tile([128, 1], f32)
            nc.scalar.activation(
                out=wdst,
                in_=wsrc,
                func=mybir.ActivationFunctionType.Relu,
                bias=wsrc[:, 0:1],
                scale=-1.0,
            )

            # stats cols: 0:2 s-sums per tile | 2:4 D_even | 4:6 D_odd
            #             | 6:8 hinge_even | 8:10 hinge_odd
            stats = small.tile([128, 10], f32)
            sqscr = small.tile([128, N, F], bf16)   # ACT square dump
            ttscr = small.tile([128, N, F2], bf16)  # TTR elementwise dump

            pc = psum.tile([1, NF], f32)
            fin = psum.tile([2, 10], f32)

            Msh = inbB[:, 0:128]
            ones = inbB[:, 128:129]
            marg = inbF[:, 2:3]

            for t in range(TPC):
                P = P2s[t]
                j0 = P[:, :, 0:F]        # rows 2p   [128, 4, 256]
                j1 = P[:, :, F:F2]       # rows 2p+1 [128, 4, 256]

                # shifted[p] = j0[p+1] = x[2p+2] for p<=126, 0 at p=127
                # (matmul moving free dim caps at 512, so split by n-pair)
                shifted = psum.tile([128, N, F], f32, tag="shift")
                for h in range(2):
                    nc.tensor.matmul(
                        shifted[:, 2 * h : 2 * h + 2, :],
                        Msh,
                        P[:, 2 * h : 2 * h + 2, 0:F],
                        start=True,
                        stop=True,
                    )
                # c partials accumulate across all four j-slices
                for h in range(2):
                    for j in range(2):
                        nc.tensor.matmul(
                            pc[:, 512 * h : 512 * h + 512],
                            ones,
                            P[:, 2 * h : 2 * h + 2, F * j : F * j + F],
                            start=(t == 0 and j == 0),
                            stop=(t == TPC - 1 and j == 1),
                            skip_group_check=True,
                        )

                # even pairs: D[2p] = ||x[2p+1] - x[2p]||^2, all 128 valid
                dE = scratch.tile([128, N, F], bf16, tag="dE")
                nc.vector.tensor_sub(dE, j1, j0)
                nc.scalar.activation(
                    out=sqscr,
                    in_=dE,
                    func=mybir.ActivationFunctionType.Square,
                    accum_out=stats[:, 2 + t : 3 + t],
                )
                # odd pairs: D[2p+1] = ||x[2p+2] - x[2p+1]||^2, p<=126
                dO = scratch.tile([128, N, F], bf16, tag="dO")
                nc.vector.tensor_sub(dO, shifted, j1)
                nc.scalar.activation(
                    out=sqscr,
                    in_=dO,
                    func=mybir.ActivationFunctionType.Square,
                    accum_out=stats[:, 4 + t : 5 + t],
                )
                # s-sums: one fused square+accumulate pass on the DVE
                # via scalar_tensor_tensor: out = (P * 1.0) * P
                nc.vector.scalar_tensor_tensor(
                    out=ttscr,
                    in0=P,
                    scalar=1.0,
                    in1=P,
                    op0=mybir.AluOpType.mult,
                    op1=mybir.AluOpType.mult,
                    accum_out=stats[:, 0 + t : 1 + t],
                )

            # hinge = relu(margin - D) for all four D columns at once
            nc.scalar.activation(
                out=stats[:, 6:10],
                in_=stats[:, 2:6],
                func=mybir.ActivationFunctionType.Relu,
                bias=marg[:, 0:1],
                scale=-1.0,
            )
            # fin[0] = unmasked partition sums, fin[1] = sums without p127
            nc.tensor.matmul(
                fin, inbF[:, 0:2], stats[:, :], start=True, stop=True
            )

            ob = small.tile([2, OUTW], f32)
            nc.vector.tensor_copy(ob[0:1, 0:NF], pc)
            nc.vector.tensor_copy(ob[0:2, NF : NF + 10], fin)
            nc.sync.dma_start(out=out[:, :], in_=ob)
    _split_multi_waits(nc)
    return nc


def _get_program() -> bass.Bass:
    global _program
    if _program is None:
        _program = _build_program()
    return _program


def _make_inb() -> np.ndarray:
    inb = np.zeros((128, 132), dtype=np.float32)
    for i in range(127):
        inb[i + 1, i] = 1.0      # shift: out[i] = in[i+1]
    inb[:, 128] = 1.0            # ones
    inb[:, 129] = 1.0            # mask
    inb[127, 129] = 0.0
    inb[:, 130] = MARGIN
    return inb


def kernel(**inputs) -> np.ndarray:
    global LAST_RESULT
    x = np.ascontiguousarray(np.asarray(inputs["x"], dtype=np.float32))
    assert x.shape == (N, S, F)
    nc = _get_program()

    inb = _make_inb()
    in_maps = []
    for k in range(NCORES):
        chunk = np.ascontiguousarray(x[:, k * LOCAL : (k + 1) * LOCAL, :])
        in_maps.append({"xc": chunk, "inb": inb})

    LAST_RESULT = run_bass_kernel_spmd(
        nc, in_maps, list(range(NCORES)), trace=TRACE
    )
    res = LAST_RESULT.results

    c = np.zeros(NF, dtype=np.float64)
    ssum = dsum = hsum = 0.0
    for r in res:
        o = r["out"].astype(np.float64)
        c += o[0, 0:NF]
        fin = o[:, NF : NF + 10]
        ssum += fin[0, 0] + fin[0, 1]
        dsum += fin[0, 2] + fin[0, 3] + fin[1, 4] + fin[1, 5]
        hsum += fin[0, 6] + fin[0, 7] + fin[1, 8] + fin[1, 9]
    gsum = float(np.sum(c * c))

    # exact host fixup for the 15 pairs straddling 256-row tile boundaries
    tb = np.arange(ROWS - 1, S - 1, ROWS)
    d = x[:, tb + 1, :].astype(np.float64) - x[:, tb, :]
    Db = (d * d).sum(axis=(0, 2))
    dsum += Db.sum()
    hsum += np.maximum(0.0, MARGIN - Db).sum()

    numerator = S * ssum - gsum - dsum + hsum
    loss = numerator / float(S * (S - 1) * 1000)
    return np.asarray(loss, dtype=np.float32)
